# revision 1
# baseline (speedup 1.0000x reference)
"""DeepAR (2-layer LSTM, HID=128) forward on 8 Trainium2 NeuronCores.

Sharding: pure data parallelism. Batch 2048 -> 256 rows per core; LSTM
weights / embedding products replicated; no cross-device communication.

Device kernel layout ("gates on partitions"):
  - The per-core batch of 256 runs as TWO independent 128-row recurrence
    chains; the Tile scheduler staggers them so the scalar (ACT) engine --
    the bottleneck (sigmoid/tanh) -- stays ~95% busy while each chain
    waits on its own matmul->sigmoid->cell->h dependency cycle.
  - State tiles are [dim=128 partitions, batch=128 free]. Gate
    pre-activations accumulate in PSUM as [128, 4*128] (chunks i,f,g,o
    along the free dim):
        psum[:, 128m:128m+128] = Wx_m^T.T @ x_t  +  Wh_m^T.T @ h_{t-1}
    with lhsT = weight chunk [K,128] stationary, rhs = state [K,128].
    NOTE: the x-MM (start=True) and h-MM (stop=True) must stay interleaved
    per chunk -- regrouping them corrupts results on HW.
  - L1's bias rides a constant ones-row appended to the input (K=66).
    L2's bias is added with one K=4 "indicator" matmul per chain.
  - g-gate rows of every weight/bias are pre-scaled by 2 on the host so a
    single Sigmoid activation covers all four gates (tanh(g) = 2*sig(2g)-1,
    fixed up on the vector engine).
  - h2 history is kept in SBUF; after the recurrence a [14,512]-chunk
    projection + stage + DMA pack + ln(1+exp) softplus + denormalization
    produce the output ([112, 3072]-packed, unpacked on the host).

kernel(**inputs) is self-contained: hardcodes shapes, shards, compiles via
bass/Tile, runs on cores 0-7 through bass_utils.run_bass_kernel_spmd, and
reassembles the full [2048, 192, 14] float32 output.
"""

import math

import numpy as np
import ml_dtypes

import concourse.bass as bass
import concourse.mybir as mybir
from concourse.tile import TileContext
from concourse import bacc, bass_utils

F32 = mybir.dt.float32
BF16 = mybir.dt.bfloat16
AF = mybir.ActivationFunctionType
ALU = mybir.AluOpType

# Model dims (hardcoded from the problem spec)
B = 2048
SEQ = 168
PRED = 24
W = SEQ + PRED          # 192
TGT = 7
TNUM = 4
TCAT = 3
MNUM = 2
MCAT = 2
COV = 4
HID = 128
INP = 65                # 7 + 28 + 4 + 26
INPX = INP + 1          # + ones row for the L1 bias
NCORES = 8
BS = B // NCORES        # 256 batch rows per core
NG = 4 * BS             # 1024: four gate chunks along psum free dim
XCH = 24                # timesteps per input-chunk DMA
NPROJ = W * BS // 512   # 96 projection chunks of [14, 512]
PGRP = 16               # chunks per partition-group in the packed output
PCC = NPROJ // PGRP     # 6 free-dim column groups

_CACHE = {}


# --------------------------------------------------------------------------
# host-side preprocessing
# --------------------------------------------------------------------------

def _host_prep(inputs):
    ge = inputs["given_enc"].astype(np.float32)
    xe = inputs["x_enc"].astype(np.float32)
    xm = inputs["x_mark_enc"].astype(np.float32)
    mx = inputs["meta_x"].astype(np.float32)
    tembs = [inputs["temb0"], inputs["temb1"], inputs["temb2"]]
    membs = [inputs["memb0"], inputs["memb1"]]

    # categorical embedding gathers
    ge_cat = [tembs[i][ge[:, :, TNUM + i].astype(np.int32)] for i in range(TCAT)]
    mx_cat = [membs[i][mx[:, MNUM + i].astype(np.int32)] for i in range(MCAT)]

    # instance norm over the time axis
    norm_mean = xe.mean(axis=1, keepdims=True)                 # [B,1,7]
    xc = xe - norm_mean
    norm_std = np.sqrt((xc * xc).mean(axis=1, keepdims=True) + 1e-5)
    xn = xc / norm_std

    # teacher forcing shift
    idx = np.clip(np.arange(W) - 1, 0, SEQ - 1)
    prev_y = xn[:, idx, :]                                     # [B,W,7]

    mx_embed = np.concatenate([mx[:, :MNUM]] + mx_cat, axis=-1)   # [B,26]
    mx_b = np.broadcast_to(mx_embed[:, None, :], (B, W, mx_embed.shape[-1]))
    inp = np.concatenate(
        [prev_y, ge[:, :, :TNUM]] + ge_cat + [xm, mx_b], axis=-1
    )                                                          # [B,W,65]
    return inp, norm_mean[:, 0, :], norm_std[:, 0, :]          # means/stds [B,7]


def _gscale(wT):
    """Scale the g-gate block (rows 2*HID:3*HID of the gate dim) by 2.
    wT is [K, 4*HID] (gate dim along columns)."""
    w = wT.copy()
    w[:, 2 * HID:3 * HID] *= 2.0
    return w


def _host_weights(inputs):
    bf = ml_dtypes.bfloat16
    w = {}
    # L1: input weights + combined bias as an extra contraction row
    wih0T = np.concatenate(
        [inputs["Wih0"].T, (inputs["bih0"] + inputs["bhh0"])[None, :]], axis=0
    )                                                          # [66, 512]
    w["wih0"] = _gscale(wih0T).astype(bf)
    w["whh0"] = _gscale(inputs["Whh0"].T).astype(bf)           # [128, 512]
    w["wih1"] = _gscale(inputs["Wih1"].T).astype(bf)           # [128, 512]
    w["whh1"] = _gscale(inputs["Whh1"].T).astype(bf)           # [128, 512]
    b2 = _gscale((inputs["bih1"] + inputs["bhh1"])[None, :])[0]  # [512]
    w["b2all"] = b2.reshape(4, HID).astype(bf)                 # [4,128]
    ind = np.zeros((4, 512), np.float32)
    for k in range(4):
        ind[k, 128 * k:128 * (k + 1)] = 1.0
    w["ind"] = ind.astype(bf)
    w["wms"] = np.concatenate([inputs["Wm"], inputs["Ws"]], axis=0).T.astype(bf)  # [128,14]
    return w


# --------------------------------------------------------------------------
# device kernel builder
# --------------------------------------------------------------------------

def build_module(nsteps=W):
    # Bacc (not raw Bass): its compile() runs move_matmul_waits_to_ldweights
    # and generate_event_semaphores, which walrus needs (max 1 wait/inst).
    nc = bacc.Bacc("TRN2", target_bir_lowering=False, debug=False,
                   enable_asserts=False, num_devices=NCORES)
    nproj = nsteps * BS // 512
    pcc = max(1, nproj // PGRP)
    ncols = nsteps * BS

    inp_d = nc.dram_tensor("inp", [INPX, ncols], BF16, kind="ExternalInput").ap()
    wih0_d = nc.dram_tensor("wih0", [INPX, 4 * HID], BF16, kind="ExternalInput").ap()
    whh0_d = nc.dram_tensor("whh0", [HID, 4 * HID], BF16, kind="ExternalInput").ap()
    wih1_d = nc.dram_tensor("wih1", [HID, 4 * HID], BF16, kind="ExternalInput").ap()
    whh1_d = nc.dram_tensor("whh1", [HID, 4 * HID], BF16, kind="ExternalInput").ap()
    b2all_d = nc.dram_tensor("b2all", [4, HID], BF16, kind="ExternalInput").ap()
    ind_d = nc.dram_tensor("ind", [4, 512], BF16, kind="ExternalInput").ap()
    wms_d = nc.dram_tensor("wms", [HID, 2 * TGT], BF16, kind="ExternalInput").ap()
    stdp_d = nc.dram_tensor("stdp", [TGT * PGRP, BS], F32, kind="ExternalInput").ap()
    meanp_d = nc.dram_tensor("meanp", [TGT * PGRP, BS], F32, kind="ExternalInput").ap()
    bsp_d = nc.dram_tensor("bsp", [TGT * PGRP, 1], F32, kind="ExternalInput").ap()

    means_d = nc.dram_tensor("means", [TGT * PGRP, 512 * pcc], F32,
                             kind="ExternalOutput").ap()
    sigmas_d = nc.dram_tensor("sigmas", [TGT * PGRP, 512 * pcc], F32,
                              kind="ExternalOutput").ap()

    with TileContext(nc) as tc:
        with tc.tile_pool(name="singles", bufs=1) as singles, \
             tc.tile_pool(name="xin", bufs=3) as xpool, \
             tc.tile_pool(name="vec", bufs=2) as vp, \
             tc.tile_pool(name="sig", bufs=2) as sigp, \
             tc.tile_pool(name="h1p", bufs=2) as h1p:

            def load(name, dram, shape, dtype=BF16):
                t = singles.tile(shape, dtype, tag=name)
                nc.sync.dma_start(out=t[:], in_=dram)
                return t

            from concourse.masks import make_identity
            ident = singles.tile([HID, HID], BF16, tag="ident")
            make_identity(nc, ident[:])
            wih0 = load("wih0", wih0_d, [INPX, 4 * HID])
            whh0 = load("whh0", whh0_d, [HID, 4 * HID])
            wih1 = load("wih1", wih1_d, [HID, 4 * HID])
            whh1 = load("whh1", whh1_d, [HID, 4 * HID])
            b2all = load("b2all", b2all_d, [4, HID])
            ind = load("ind", ind_d, [4, 512])
            wms = load("wms", wms_d, [HID, 2 * TGT])
            stdp = load("stdp", stdp_d, [TGT * PGRP, BS], F32)
            meanp = load("meanp", meanp_d, [TGT * PGRP, BS], F32)
            bsp = load("bsp", bsp_d, [TGT * PGRP, 1], F32)

            h2_hist = singles.tile([HID, ncols], BF16, tag="h2_hist")
            means_sb = singles.tile([TGT * PGRP, 512 * pcc], F32, tag="means_sb")
            sigraw_sb = singles.tile([TGT * PGRP, 512 * pcc], F32, tag="sigraw_sb")

            # Two independent batch chains of 128 rows each. Their
            # recurrences never interact, so the Tile scheduler staggers
            # them to hide per-step dependency-chain latency.
            CB = BS // 2   # 128 rows per chain
            NGc = 4 * CB   # 512: per-chain gate psum width (1 bank)
            state = [dict(h1=None, c1=None, c2=None) for _ in range(2)]
            x_tile = None

            # One shared 3-buf PSUM pool per chain hosts both g1 and g2
            # tiles (same [128,512] shape) -> 6 banks, leaving 2 banks for
            # the projection pool so the output projection + pack overlap
            # the recurrence instead of running as a serial tail.
            pools = {}
            import contextlib
            ctx = contextlib.ExitStack()
            for ch in (0, 1):
                pools[ch] = ctx.enter_context(
                    tc.tile_pool(name=f"pg{ch}", bufs=3, space="PSUM"))
            projp = ctx.enter_context(
                tc.tile_pool(name="proj", bufs=2, space="PSUM"))
            stagep = ctx.enter_context(tc.tile_pool(name="stage", bufs=3))

            def cell(t, g, cprev, prefix):
                """Gate nonlinearities + cell update for one chain/layer."""
                s = sigp.tile([HID, NGc], BF16, tag=f"s{prefix}")
                nc.scalar.activation(s[:], g[:], AF.Sigmoid)
                si, sf = s[:, 0:CB], s[:, CB:2 * CB]
                sg, so = s[:, 2 * CB:3 * CB], s[:, 3 * CB:4 * CB]
                gt = vp.tile([HID, CB], BF16, tag=f"gt{prefix}")
                nc.vector.tensor_scalar(gt[:], sg, 2.0, 1.0,
                                        ALU.mult, ALU.subtract)
                u = vp.tile([HID, CB], BF16, tag=f"u{prefix}")
                nc.vector.tensor_mul(u[:], si, gt[:])
                if t == 0:
                    c = u
                else:
                    v = vp.tile([HID, CB], BF16, tag=f"v{prefix}")
                    nc.vector.tensor_mul(v[:], sf, cprev[:])
                    c = vp.tile([HID, CB], BF16, tag=f"c{prefix}")
                    nc.vector.tensor_add(c[:], u[:], v[:])
                tt = vp.tile([HID, CB], BF16, tag=f"t{prefix}")
                nc.scalar.activation(tt[:], c[:], AF.Tanh)
                return so, tt, c

            for t in range(nsteps):
                if t % XCH == 0:
                    nx = min(XCH, nsteps - t)
                    x_tile = xpool.tile([INPX, XCH * BS], BF16, tag="x")
                    nc.sync.dma_start(
                        out=x_tile[:, :nx * BS],
                        in_=inp_d[:, t * BS:(t + nx) * BS])
                xo = (t % XCH) * BS

                for ch in (0, 1):
                    st = state[ch]
                    xt = x_tile[:, xo + CB * ch:xo + CB * (ch + 1)]

                    # ---- layer 1: x-parts first (independent of h1), the
                    # h1-dependent matmuls last so the sigma unblocks early
                    g1 = pools[ch].tile([HID, NGc], F32, tag=f"pg{ch}")
                    for m in range(4):
                        sl = slice(CB * m, CB * (m + 1))
                        nc.tensor.matmul(g1[:, sl],
                                         wih0[:, HID * m:HID * (m + 1)], xt,
                                         start=True, stop=(t == 0))
                        if t > 0:
                            nc.tensor.matmul(g1[:, sl],
                                             whh0[:, HID * m:HID * (m + 1)],
                                             st["h1"][:],
                                             start=False, stop=True)
                    so1, tt1, c1 = cell(t, g1, st["c1"], f"1{ch}")
                    h1 = h1p.tile([HID, CB], BF16, tag=f"h1{ch}")
                    nc.vector.tensor_mul(h1[:], so1, tt1[:])
                    st["c1"] = c1
                    st["h1"] = h1

                    # ---- layer 2: bias + h-part first, x-part (h1-dep) last
                    g2 = pools[ch].tile([HID, NGc], F32, tag=f"pg{ch}")
                    nc.tensor.matmul(g2[:], b2all[:], ind[:],
                                     start=True, stop=False)
                    hoff = t * BS + CB * ch
                    poff = (t - 1) * BS + CB * ch
                    for m in range(4):
                        sl = slice(CB * m, CB * (m + 1))
                        # h-part first (h2(t-1) is ready early); the
                        # h1-dependent x-part last so sigma2 unblocks sooner
                        if t > 0:
                            nc.tensor.matmul(
                                g2[:, sl], whh1[:, HID * m:HID * (m + 1)],
                                h2_hist[:, poff:poff + CB],
                                start=False, stop=False)
                        nc.tensor.matmul(g2[:, sl],
                                         wih1[:, HID * m:HID * (m + 1)], h1[:],
                                         start=False, stop=(m == 3))
                    so2, tt2, c2 = cell(t, g2, st["c2"], f"2{ch}")
                    nc.vector.tensor_mul(h2_hist[:, hoff:hoff + CB],
                                         so2, tt2[:])
                    st["c2"] = c2

                # ---- output projection + pack for steps (t-1, t),
                # overlapped with the recurrence (both chains' h2 written)
                if t % 2 == 1:
                    c = t // 2
                    pp = projp.tile([2 * TGT, 512], F32, tag="pp")
                    nc.tensor.matmul(pp[:], wms[:],
                                     h2_hist[:, 512 * c:512 * (c + 1)],
                                     start=True, stop=True)
                    g = c % PGRP
                    cc = c // PGRP
                    dst = slice(512 * cc, 512 * (cc + 1))
                    # DMA cannot read PSUM, and compute engines cannot write
                    # at unaligned partition bases -> stage at partition 0,
                    # then DMA into the packed layout.
                    stt = stagep.tile([2 * TGT, 512], F32, tag="st")
                    if c % 2 == 0:
                        nc.scalar.copy(stt[:], pp[:])
                    else:
                        nc.vector.tensor_copy(stt[:], pp[:])
                    nc.sync.dma_start(
                        out=means_sb[TGT * g:TGT * (g + 1), dst],
                        in_=stt[0:TGT, :])
                    nc.sync.dma_start(
                        out=sigraw_sb[TGT * g:TGT * (g + 1), dst],
                        in_=stt[TGT:2 * TGT, :])
            ctx.close()

            if True:
                # ---- epilogue: softplus + denorm ----
                nf = 2 * pcc  # broadcast factor along free dim
                std_bc = stdp[:, :].unsqueeze(1).broadcast_to(
                    [TGT * PGRP, nf, BS])
                mean_bc = meanp[:, :].unsqueeze(1).broadcast_to(
                    [TGT * PGRP, nf, BS])
                # softplus(x+bs) = ln(1 + exp(x+bs)); Softplus itself has no
                # ACT table set, but exp and ln share one.
                sigsp = singles.tile([TGT * PGRP, 512 * pcc], F32, tag="sigsp")
                nc.scalar.activation(sigsp[:], sigraw_sb[:], AF.Exp,
                                     bias=bsp[:, :])
                nc.scalar.activation(sigsp[:], sigsp[:], AF.Ln, bias=1.0)
                nc.vector.tensor_mul(sigsp[:], sigsp[:], std_bc)
                nc.vector.tensor_mul(means_sb[:], means_sb[:], std_bc)
                nc.vector.tensor_add(means_sb[:], means_sb[:], mean_bc)
                nc.sync.dma_start(out=means_d, in_=means_sb[:])
                nc.sync.dma_start(out=sigmas_d, in_=sigsp[:])

    nc.finalize()
    return nc


# --------------------------------------------------------------------------
# top-level entry
# --------------------------------------------------------------------------

def _pack_norm(arr):
    """[BS,7] per-core norm stats -> [112, BS] tiled PGRP times."""
    a = arr.T.astype(np.float32)                 # [7, BS]
    return np.tile(a, (PGRP, 1)).astype(np.float32)


def run(inputs, trace=False, nsteps=W):
    inputs = {k: np.asarray(v) for k, v in inputs.items()}
    inp, nmean, nstd = _host_prep(inputs)
    wts = _host_weights(inputs)
    bf = ml_dtypes.bfloat16

    bm = inputs["bm"].astype(np.float32)
    bs_ = inputs["bs"].astype(np.float32)

    in_maps = []
    for k in range(NCORES):
        bsl = slice(k * BS, (k + 1) * BS)
        # [BS, nsteps, 66] -> [66, nsteps*BS] with col = t*BS + b
        xi = np.concatenate(
            [inp[bsl, :nsteps, :], np.ones((BS, nsteps, 1), np.float32)],
            axis=-1)
        xiT = np.ascontiguousarray(xi.transpose(2, 1, 0).reshape(INPX, -1))
        std_c = nstd[bsl]                        # [BS, 7]
        mean_c = nmean[bsl]
        m = dict(wts)
        m["inp"] = xiT.astype(bf)
        m["stdp"] = _pack_norm(std_c)
        # fold bm*std + mean into the additive term
        m["meanp"] = _pack_norm(bm[None, :] * std_c + mean_c)
        m["bsp"] = np.tile(bs_, PGRP)[:, None].astype(np.float32)
        in_maps.append(m)

    key = nsteps
    if key not in _CACHE:
        _CACHE[key] = build_module(nsteps)
    nc = _CACHE[key]

    res = bass_utils.run_bass_kernel_spmd(
        nc, in_maps, core_ids=list(range(NCORES)), trace=False)

    nproj = nsteps * BS // 512
    pcc = max(1, nproj // PGRP)
    out = np.empty((B, nsteps, 2 * TGT), np.float32)
    for k in range(NCORES):
        r = res.results[k]
        for name, off in (("means", 0), ("sigmas", TGT)):
            a = r[name].reshape(PGRP, TGT, pcc, 2, BS)
            # [g, o, cc, tau, b] -> [b, cc, g, tau, o]
            a = a.transpose(4, 2, 0, 3, 1).reshape(BS, nsteps, TGT)
            out[k * BS:(k + 1) * BS, :, off:off + TGT] = a
    return out, res.exec_time_ns


def kernel(**inputs):
    out, _ = run(inputs, trace=False)
    return out



# revision 19
# speedup vs baseline: 1.1636x; 1.1636x over previous
"""DeepAR (2-layer LSTM, HID=128) forward on 8 Trainium2 NeuronCores.

Sharding: pure data parallelism. Batch 2048 -> 256 rows per core, run as two
128-row chains; LSTM weights replicated; no cross-device communication.

Device kernel ("ride-along sigmoids + fp8 DoubleRow recurrence"):
  - ACT is the bottleneck engine, so the kernel issues exactly TWO sigmoid
    instructions per chain per step and nothing else on ACT:
      I_A(t) = sigmoid[ g1(t) gates (4*128) | C2(t-2) ride (128) ]
      I_B(t) = sigmoid[ g2(t-1) gates      | C1(t) ride        ]
    Layer 2 runs one step behind layer 1. The cell state is kept doubled
    (C = 2c) so one sigmoid covers both the gates (g-rows of the weights are
    pre-doubled: tanh(g) = 2*sig(2g)-1) and the tanh rides
    (tanh(c) = 2*sig(C)-1). No standalone Tanh instructions exist.
  - The recurrent state enters the matmuls as m = h/2 = so*(sig(C)-0.5),
    one fused scalar_tensor_tensor on DVE, written directly as fp8.
    Each gate chunk is then ONE DoubleRow fp8 matmul with virtual K=256:
      layer1: rhs = [x(t) | m1(t-1)],   lhsT = [Wih0 | 2*Whh0]
      layer2: rhs = [m1(t-1) | m2(t-2)], lhsT = [2*Wih1 | 2*Whh1]
    x / m1 / m2 live as adjacent 128-col regions of one per-slot layout
    [x(s) | m1(s-1) | m2(s-2)] inside XCH-slot chunk tiles, so both layers'
    rhs pairs are contiguous 3D APs of the same tile, and the projection
    reads 4-slot m2 spans.
  - Cell update per layer on DVE (gt = 4*sig(2g)-2, u = si*gt, C = u+v) with
    v = sf*C_prev on GpSimd; the psum ride region is filled by two identity
    matmuls (I@u + I@v) on the tensor engine.
  - One 2-bank PSUM slot per chain holds I_A then I_B alternately (the
    pipeline's data deps already serialize the reuse); projection psum and
    staging run incrementally; means/sigma post-processing runs per 512-col
    block (means on GpSimd, softplus Exp/Ln on ACT) as blocks complete.

kernel(**inputs) is self-contained: hardcodes shapes, shards, compiles via
bass/Tile, runs on cores 0-7 through bass_utils.run_bass_kernel_spmd, and
reassembles the full [2048, 192, 14] float32 output.
"""

import numpy as np
import ml_dtypes

import concourse.bass as bass
import concourse.mybir as mybir
from concourse.tile import TileContext
from concourse import bacc, bass_utils

F32 = mybir.dt.float32
BF16 = mybir.dt.bfloat16
F8 = mybir.dt.float8e4
AF = mybir.ActivationFunctionType
ALU = mybir.AluOpType
DR = mybir.MatmulPerfMode.DoubleRow

f8 = ml_dtypes.float8_e4m3
bf = ml_dtypes.bfloat16

# Model dims (hardcoded from the problem spec)
B = 2048
SEQ = 168
PRED = 24
W = SEQ + PRED          # 192
TGT = 7
TNUM = 4
TCAT = 3
MNUM = 2
MCAT = 2
COV = 4
HID = 128
INP = 65                # 7 + 28 + 4 + 26
INPX = INP + 1          # + ones row for the L1 bias
NCORES = 8
BS = B // NCORES        # 256 batch rows per core
CB = 128                # chain batch (2 chains per core)
XCH = 28                # slots per chunk tile
GRP = 4                 # slots per projection group
PGRP = 16               # groups per packed partition block (16*7 = 112)

_CACHE = {}


def _plan(nsteps):
    slots = nsteps + 2                    # slot s: [x(s) | m1(s-1) | m2(s-2)]
    slots_pad = -(-slots // XCH) * XCH
    nchunks = slots_pad // XCH
    ngrp = slots_pad // GRP               # per-chain projection groups
    ngrp_tot = 2 * ngrp                   # global groups G = 2*g + ch
    nblk = -(-ngrp_tot // PGRP)           # 512-col blocks in the packed sb
    return slots, slots_pad, nchunks, ngrp, ngrp_tot, nblk


# --------------------------------------------------------------------------
# host-side preprocessing
# --------------------------------------------------------------------------

def _host_prep(inputs):
    ge = inputs["given_enc"].astype(np.float32)
    xe = inputs["x_enc"].astype(np.float32)
    xm = inputs["x_mark_enc"].astype(np.float32)
    mx = inputs["meta_x"].astype(np.float32)
    tembs = [inputs["temb0"], inputs["temb1"], inputs["temb2"]]
    membs = [inputs["memb0"], inputs["memb1"]]

    ge_cat = [tembs[i][ge[:, :, TNUM + i].astype(np.int32)] for i in range(TCAT)]
    mx_cat = [membs[i][mx[:, MNUM + i].astype(np.int32)] for i in range(MCAT)]

    norm_mean = xe.mean(axis=1, keepdims=True)                 # [B,1,7]
    xc = xe - norm_mean
    norm_std = np.sqrt((xc * xc).mean(axis=1, keepdims=True) + 1e-5)
    xn = xc / norm_std

    idx = np.clip(np.arange(W) - 1, 0, SEQ - 1)
    prev_y = xn[:, idx, :]                                     # [B,W,7]

    mx_embed = np.concatenate([mx[:, :MNUM]] + mx_cat, axis=-1)   # [B,26]
    mx_b = np.broadcast_to(mx_embed[:, None, :], (B, W, mx_embed.shape[-1]))
    inp = np.concatenate(
        [prev_y, ge[:, :, :TNUM]] + ge_cat + [xm, mx_b], axis=-1
    )                                                          # [B,W,65]
    return inp, norm_mean[:, 0, :], norm_std[:, 0, :]          # [B,7] stats


def _gscale(wT):
    """Double the g-gate block (cols 2H:3H of the gate dim). wT is [K, 4H]."""
    w = wT.copy()
    w[:, 2 * HID:3 * HID] *= 2.0
    return w


def _host_weights(inputs):
    w = {}
    # L1 DoubleRow weights [128, 2, 512]: half0 = Wih0^T (+bias row), half1 = 2*Whh0^T
    w1 = np.zeros((HID, 2, 4 * HID), np.float32)
    w1[:INPX, 0, :] = _gscale(np.concatenate(
        [inputs["Wih0"].T, (inputs["bih0"] + inputs["bhh0"])[None, :]], axis=0))
    w1[:, 1, :] = _gscale(2.0 * inputs["Whh0"].T)
    w["w1dr"] = w1.astype(f8)
    # L2 DoubleRow weights: half0 = 2*Wih1^T, half1 = 2*Whh1^T
    w2 = np.zeros((HID, 2, 4 * HID), np.float32)
    w2[:, 0, :] = _gscale(2.0 * inputs["Wih1"].T)
    w2[:, 1, :] = _gscale(2.0 * inputs["Whh1"].T)
    w["w2dr"] = w2.astype(f8)
    # L2 bias via indicator matmul
    b2 = _gscale((inputs["bih1"] + inputs["bhh1"])[None, :])[0]
    w["b2all"] = b2.reshape(4, HID).astype(bf)
    ind = np.zeros((4, 4 * HID), np.float32)
    for k in range(4):
        ind[k, HID * k:HID * (k + 1)] = 1.0
    w["ind"] = ind.astype(bf)
    # projection: h2 = 2*m2 -> lhsT = [2*Wm; 2*Ws]^T  [128, 14]
    w["wproj"] = (2.0 * np.concatenate([inputs["Wm"], inputs["Ws"]], axis=0).T
                  ).astype(bf)
    return w


# --------------------------------------------------------------------------
# device kernel builder
# --------------------------------------------------------------------------

def build_module(nsteps=W, no_proj=False):
    slots, slots_pad, nchunks, ngrp, ngrp_tot, nblk = _plan(nsteps)
    nc = bacc.Bacc("TRN2", target_bir_lowering=False, debug=False,
                   enable_asserts=False, num_devices=NCORES)

    xin_d = [nc.dram_tensor(f"xin{ch}", [INPX, nsteps * CB], F8,
                            kind="ExternalInput").ap() for ch in (0, 1)]
    w1dr_d = nc.dram_tensor("w1dr", [HID, 2 * 4 * HID], F8, kind="ExternalInput").ap()
    w2dr_d = nc.dram_tensor("w2dr", [HID, 2 * 4 * HID], F8, kind="ExternalInput").ap()
    b2all_d = nc.dram_tensor("b2all", [4, HID], BF16, kind="ExternalInput").ap()
    ind_d = nc.dram_tensor("ind", [4, 4 * HID], BF16, kind="ExternalInput").ap()
    wproj_d = nc.dram_tensor("wproj", [HID, 2 * TGT], BF16, kind="ExternalInput").ap()
    stdm_d = nc.dram_tensor("stdm", [TGT * PGRP, 512 * nblk], F32,
                            kind="ExternalInput").ap()
    madd_d = nc.dram_tensor("madd", [TGT * PGRP, 512 * nblk], F32,
                            kind="ExternalInput").ap()
    bsp_d = nc.dram_tensor("bsp", [TGT * PGRP, 1], F32, kind="ExternalInput").ap()

    means_d = nc.dram_tensor("means", [TGT * PGRP, 512 * nblk], F32,
                             kind="ExternalOutput").ap()
    sigmas_d = nc.dram_tensor("sigmas", [TGT * PGRP, 512 * nblk], F32,
                              kind="ExternalOutput").ap()

    with TileContext(nc) as tc:
        import contextlib
        ctx = contextlib.ExitStack()
        singles = ctx.enter_context(tc.tile_pool(name="singles", bufs=1))
        xmp = {ch: ctx.enter_context(tc.tile_pool(name=f"xm{ch}", bufs=3))
               for ch in (0, 1)}
        sap = {ch: ctx.enter_context(tc.tile_pool(name=f"sa{ch}", bufs=2))
               for ch in (0, 1)}
        sbp = {ch: ctx.enter_context(tc.tile_pool(name=f"sb{ch}", bufs=2))
               for ch in (0, 1)}
        vecp = ctx.enter_context(tc.tile_pool(name="vec", bufs=2))
        pgp = {ch: ctx.enter_context(
            tc.tile_pool(name=f"pg{ch}", bufs=1, space="PSUM"))
            for ch in (0, 1)}
        projp = ctx.enter_context(tc.tile_pool(name="proj", bufs=2, space="PSUM"))
        stagep = ctx.enter_context(tc.tile_pool(name="stage", bufs=3))

        def load(name, dram, shape, dtype=BF16):
            t = singles.tile(shape, dtype, tag=name, name=name)
            nc.sync.dma_start(out=t[:], in_=dram)
            return t

        from concourse.masks import make_identity
        ident = singles.tile([HID, HID], BF16, tag="ident", name="ident")
        make_identity(nc, ident[:])
        # 4*I: rides feed u/4, the matmul scales it back (saves a DVE hop)
        ident4 = singles.tile([HID, HID], BF16, tag="ident4", name="ident4")
        nc.vector.tensor_scalar(ident4[:], ident[:], 4.0, None, ALU.mult)
        w1dr = load("w1dr", w1dr_d, [HID, 2, 4 * HID], F8)
        w2dr = load("w2dr", w2dr_d, [HID, 2, 4 * HID], F8)
        b2all = load("b2all", b2all_d, [4, HID])
        ind = load("ind", ind_d, [4, 4 * HID])
        wproj = load("wproj", wproj_d, [HID, 2 * TGT])
        stdm = load("stdm", stdm_d, [TGT * PGRP, 512 * nblk], F32)
        madd = load("madd", madd_d, [TGT * PGRP, 512 * nblk], F32)
        bsp = load("bsp", bsp_d, [TGT * PGRP, 1], F32)

        means_sb = singles.tile([TGT * PGRP, 512 * nblk], F32, tag="means_sb",
                                name="means_sb")
        sigraw_sb = singles.tile([TGT * PGRP, 512 * nblk], F32, tag="sigraw_sb",
                                 name="sigraw_sb")

        # chunk tiles: [128, XCH, 3, 128] fp8; slot s: [x(s) | m1(s-1) | m2(s-2)]
        chunk = {0: {}, 1: {}}

        def get_chunk(ch, ci):
            if ci not in chunk[ch]:
                xt = xmp[ch].tile([HID, XCH, 3, CB], F8, tag="xm",
                                  name=f"xm{ch}_{ci}")
                chunk[ch][ci] = xt
                if ci < 3:
                    # one-time per physical buffer: zero the x rows the DMA
                    # never writes (their weights are zero, but stale NaNs
                    # would still poison the fp8 matmul). Partition base must
                    # be 32-aligned; rows 64-65 get re-written by the DMA.
                    nc.gpsimd.memset(xt[64:, :, 0, :], 0.0)
                if ci == 0:
                    nc.gpsimd.memset(xt[:, 0, 1:3, :], 0.0)   # m1(-1), m2(-2)
                    nc.gpsimd.memset(xt[:, 1, 2, :], 0.0)     # m2(-1)
                if ci == nchunks - 1 and slots - ci * XCH < XCH:
                    nc.gpsimd.memset(xt[:, slots - ci * XCH:, 2, :], 0.0)
                s0 = ci * XCH
                nx = min(XCH, nsteps - s0)
                if nx > 0:
                    nc.sync.dma_start(
                        out=xt[:INPX, 0:nx, 0, :],
                        in_=xin_d[ch][:, s0 * CB:(s0 + nx) * CB])
            return chunk[ch][ci]

        def slot(ch, s):
            return get_chunk(ch, s // XCH)[:, s % XCH, :, :]

        state = [dict(c1=None, c2=None, sA=None, sB=None) for _ in (0, 1)]
        pg = {ch: pgp[ch].tile([HID, 640], F32, tag="pg", name=f"pg{ch}")
              for ch in (0, 1)}

        # ---- projection / staging / epilogue ----
        commit_q = []
        grp_done = [0, 0]
        blk_parts = [0] * nblk
        pool_q = []          # deferred small gpsimd ops: (fn,) chopped pieces

        def emit_block(cc):
            """Means post-processing for a finished 512-col block, chopped
            into small gpsimd pieces so the cell's v-muls never queue behind
            a long block op. Sigma softplus is deferred to the tail."""
            d0 = 512 * cc
            for p in range(4):
                d = slice(d0 + 128 * p, d0 + 128 * (p + 1))
                pool_q.append(lambda d=d: nc.gpsimd.tensor_tensor(
                    means_sb[:, d], means_sb[:, d], stdm[:, d], ALU.mult))
                pool_q.append(lambda d=d: nc.gpsimd.tensor_tensor(
                    means_sb[:, d], means_sb[:, d], madd[:, d], ALU.add))
            dd = slice(d0, d0 + 512)
            pool_q.append(lambda dd=dd: nc.sync.dma_start(
                out=means_d[:, dd], in_=means_sb[:, dd]))

        def drain_pool_q(k=2):
            for _ in range(min(k, len(pool_q))):
                pool_q.pop(0)()

        def emit_group(ch, g):
            """Project slots [4g, 4g+4) of chain ch, stage + pack."""
            G = 2 * g + ch
            ci = (GRP * g) // XCH
            xt = chunk[ch][ci]
            so = (GRP * g) % XCH
            pp = projp.tile([2 * TGT, 512], F32, tag="pp", name=f"pp{G}")
            nc.tensor.matmul(pp[:], wproj[:], xt[:, so:so + GRP, 2, :],
                             start=True, stop=True)
            st = stagep.tile([2 * TGT, 512], F32, tag="st", name=f"st{G}")
            nc.vector.tensor_copy(st[:], pp[:])
            prow = TGT * (G % PGRP)
            cc = G // PGRP
            dst = slice(512 * cc, 512 * (cc + 1))
            nc.sync.dma_start(out=means_sb[prow:prow + TGT, dst], in_=st[:TGT, :])
            nc.sync.dma_start(out=sigraw_sb[prow:prow + TGT, dst], in_=st[TGT:, :])
            blk_parts[cc] += 1
            if blk_parts[cc] == min(PGRP, ngrp_tot - cc * PGRP):
                emit_block(cc)

        def maybe_proj(ch, t):
            if no_proj:
                return
            # group g's last m2 (slot 4g+3) is written during tick 4g+3
            while grp_done[ch] < ngrp and t >= GRP * grp_done[ch] + GRP - 1:
                emit_group(ch, grp_done[ch])
                grp_done[ch] += 1

        def tail_sigma():
            """Bulk softplus + denorm of the packed sigma rows, then DMA."""
            n = 512 * nblk
            h = n // 2
            for d in (slice(0, h), slice(h, n)):
                nc.scalar.activation(sigraw_sb[:, d], sigraw_sb[:, d], AF.Exp,
                                     bias=bsp[:, :])
                nc.scalar.activation(sigraw_sb[:, d], sigraw_sb[:, d], AF.Ln,
                                     bias=1.0)
                nc.vector.tensor_mul(sigraw_sb[:, d], sigraw_sb[:, d],
                                     stdm[:, d])
                nc.sync.dma_start(out=sigmas_d[:, d], in_=sigraw_sb[:, d])

        # ---- per-step pieces ----
        def cell(ch, layer, sgt, first):
            """Cell update. u4 = u/4 = (sig(2g)-0.5)*si comes straight off the
            sigma outputs (no chained DVE ops on the ride path); v on GpSimd;
            C = 4*u4 + v in one fused op off the critical path."""
            st = state[ch]
            si, sf = sgt[:, 0:CB], sgt[:, CB:2 * CB]
            s2g = sgt[:, 2 * CB:3 * CB]
            u4 = vecp.tile([HID, CB], BF16, tag=f"u{layer}{ch}",
                           name=f"u{layer}{ch}")
            nc.vector.scalar_tensor_tensor(u4[:], s2g, 0.5, si,
                                           ALU.subtract, ALU.mult)
            if first:
                v = None
            else:
                v = vecp.tile([HID, CB], BF16, tag=f"v{layer}{ch}",
                              name=f"v{layer}{ch}")
                nc.gpsimd.tensor_tensor(v[:], sf, st[f"c{layer}"][:], ALU.mult)
            # C = 4*u4 + v is deferred to the tick end (slack: only the next
            # tick's v-mul reads it) so it never delays the next m/u4 pair
            commit_q.append((ch, layer, u4, v))
            return u4, v

        def cell_commit():
            while commit_q:
                ch, layer, u4, v = commit_q.pop(0)
                c_new = vecp.tile([HID, CB], BF16, tag=f"c{layer}{ch}",
                                  name=f"c{layer}{ch}")
                if v is None:
                    nc.vector.tensor_scalar(c_new[:], u4[:], 4.0, None,
                                            ALU.mult)
                else:
                    nc.vector.scalar_tensor_tensor(c_new[:], u4[:], 4.0, v[:],
                                                   ALU.mult, ALU.add)
                state[ch][f"c{layer}"] = c_new

        def ride(ch, u4, v, lo=512):
            nc.tensor.matmul(pg[ch][:, lo:lo + CB], ident4[:], u4[:],
                             start=True, stop=(v is None))
            if v is not None:
                nc.tensor.matmul(pg[ch][:, lo:lo + CB], ident[:], v[:],
                                 start=False, stop=True)

        def m_stt(dst, s_c, s_o):
            """dst(fp8) = (sig(C) - 0.5) * so   [= h/2]"""
            nc.vector.scalar_tensor_tensor(dst, s_c, 0.5, s_o,
                                           ALU.subtract, ALU.mult)

        def l2_gates(ch, t):
            """bias + 4 DoubleRow matmuls for g2(t-1) into pg[ch][0:512]."""
            sl = slot(ch, t)
            nc.tensor.matmul(pg[ch][:, 0:512], b2all[:], ind[:],
                             start=True, stop=False)
            for m in range(4):
                nc.tensor.matmul(pg[ch][:, CB * m:CB * (m + 1)],
                                 w2dr[:, :, HID * m:HID * (m + 1)],
                                 sl[:, 1:3, :], start=False, stop=True,
                                 perf_mode=DR)

        def l1_gates(ch, t):
            """4 DoubleRow matmuls for g1(t) into pg[ch][0:512]."""
            get_chunk(ch, min((t + 4) // XCH, nchunks - 1))
            sl = slot(ch, t)
            for m in range(4):
                nc.tensor.matmul(pg[ch][:, CB * m:CB * (m + 1)],
                                 w1dr[:, :, HID * m:HID * (m + 1)],
                                 sl[:, 0:2, :], start=True, stop=True,
                                 perf_mode=DR)

        # ---------------- main loop ----------------
        # prologue: gate matmuls for g1(0) (later ticks emit them in half B)
        for ch in (0, 1):
            l1_gates(ch, 0)

        for t in range(nsteps + 1):
            # ---- half A ----
            for ch in (0, 1):
                st = state[ch]
                if t < nsteps:
                    hi = 640 if t >= 2 else 512
                    sA = sap[ch].tile([HID, 640], BF16, tag="sA",
                                      name=f"sA{ch}_{t}")
                    nc.scalar.activation(sA[:, :hi], pg[ch][:, :hi], AF.Sigmoid)
                    if t >= 2:
                        # m2(t-2) -> slot t region 2 ; so2(t-2) is in sB(t-1)
                        m_stt(slot(ch, t)[:, 2, :], sA[:, 512:640],
                              st["sB"][:, 3 * CB:4 * CB])
                    if t >= 1:
                        l2_gates(ch, t)       # PE: ready before the rides
                    u1, v1 = cell(ch, 1, sA, t == 0)
                    st["sA"] = sA
                    ride(ch, u1, v1)
                else:
                    # tail: the C2(nsteps-2) ride was written at t-1 half B;
                    # sigma it alone, then the last L2 gates
                    sA = sap[ch].tile([HID, 640], BF16, tag="sA",
                                      name=f"sA{ch}_{t}")
                    nc.scalar.activation(sA[:, 512:640], pg[ch][:, 512:640],
                                         AF.Sigmoid)
                    m_stt(slot(ch, t)[:, 2, :], sA[:, 512:640],
                          st["sB"][:, 3 * CB:4 * CB])
                    st["sA"] = sA
                    l2_gates(ch, t)

            # ---- half B ----
            for ch in (0, 1):
                st = state[ch]
                sB = sbp[ch].tile([HID, 640], BF16, tag="sB", name=f"sB{ch}_{t}")
                if t == 0:
                    nc.scalar.activation(sB[:, 512:640], pg[ch][:, 512:640],
                                         AF.Sigmoid)
                elif t == nsteps:
                    nc.scalar.activation(sB[:, 0:512], pg[ch][:, 0:512],
                                         AF.Sigmoid)
                else:
                    nc.scalar.activation(sB[:], pg[ch][:], AF.Sigmoid)
                if t < nsteps:
                    # m1(t) -> slot t+1 region 1 ; so1(t) is in sA(t)
                    m_stt(slot(ch, t + 1)[:, 1, :], sB[:, 512:640],
                          st["sA"][:, 3 * CB:4 * CB])
                    l1_gates(ch, t + 1)       # PE: before the rides
                if t >= 1:
                    u2, v2 = cell(ch, 2, sB, t == 1)
                    ride(ch, u2, v2)   # C2(t-1) ride -> next sigma's region
                st["sB"] = sB

            # ---- deferred/slack work at tick end ----
            cell_commit()
            for ch in (0, 1):
                maybe_proj(ch, t)
            drain_pool_q()

            if t == nsteps:
                # final ride: sigma C2(nsteps-1), emit m2(nsteps-1), flush
                for ch in (0, 1):
                    st = state[ch]
                    sF = sap[ch].tile([HID, 640], BF16, tag="sA",
                                      name=f"sF{ch}")
                    nc.scalar.activation(sF[:, 512:640], pg[ch][:, 512:640],
                                         AF.Sigmoid)
                    m_stt(slot(ch, t + 1)[:, 2, :], sF[:, 512:640],
                          st["sB"][:, 3 * CB:4 * CB])
                    maybe_proj(ch, 10 ** 9)
                drain_pool_q(10 ** 9)
                if not no_proj:
                    tail_sigma()

        ctx.close()

    nc.finalize()
    return nc


# --------------------------------------------------------------------------
# top-level entry
# --------------------------------------------------------------------------

def run(inputs, trace=False, nsteps=W):
    inputs = {k: np.asarray(v) for k, v in inputs.items()}
    slots, slots_pad, nchunks, ngrp, ngrp_tot, nblk = _plan(nsteps)
    inp, nmean, nstd = _host_prep(inputs)
    wts = _host_weights(inputs)
    bm = inputs["bm"].astype(np.float32)
    bs_ = inputs["bs"].astype(np.float32)

    in_maps = []
    for k in range(NCORES):
        m = {kk: (vv.reshape(vv.shape[0], -1) if vv.ndim == 3 else vv)
             for kk, vv in wts.items()}
        for ch in (0, 1):
            bsl = slice(k * BS + ch * CB, k * BS + ch * CB + CB)
            xi = np.concatenate(
                [inp[bsl, :nsteps, :], np.ones((CB, nsteps, 1), np.float32)],
                axis=-1)                                   # [CB, T, 66]
            m[f"xin{ch}"] = np.ascontiguousarray(
                xi.transpose(2, 1, 0).reshape(INPX, -1)).astype(f8)
        stdm = np.zeros((TGT * PGRP, 512 * nblk), np.float32)
        madd = np.zeros((TGT * PGRP, 512 * nblk), np.float32)
        for G in range(ngrp_tot):
            ch = G % 2
            prow = TGT * (G % PGRP)
            c0 = 512 * (G // PGRP)
            bsl = slice(k * BS + ch * CB, k * BS + ch * CB + CB)
            std_c = nstd[bsl]      # [CB, 7]
            mean_c = nmean[bsl]
            for j in range(GRP):
                cs = slice(c0 + CB * j, c0 + CB * (j + 1))
                stdm[prow:prow + TGT, cs] = std_c.T
                madd[prow:prow + TGT, cs] = bm[:, None] * std_c.T + mean_c.T
        m["stdm"] = stdm
        m["madd"] = madd
        m["bsp"] = np.tile(bs_, PGRP)[:, None].astype(np.float32)
        in_maps.append(m)

    key = nsteps
    if key not in _CACHE:
        _CACHE[key] = build_module(nsteps)
    nc = _CACHE[key]

    res = bass_utils.run_bass_kernel_spmd(
        nc, in_maps, core_ids=list(range(NCORES)), trace=False)

    out = np.empty((B, nsteps, 2 * TGT), np.float32)
    for k in range(NCORES):
        r = res.results[k]
        for name, off in (("means", 0), ("sigmas", TGT)):
            a = r[name]                      # [112, 512*nblk]
            for G in range(ngrp_tot):
                ch, g = G % 2, G // 2
                prow = TGT * (G % PGRP)
                c0 = 512 * (G // PGRP)
                for j in range(GRP):
                    tau = GRP * g + j - 2
                    if tau < 0 or tau >= nsteps:
                        continue
                    blk = a[prow:prow + TGT, c0 + CB * j:c0 + CB * (j + 1)]
                    out[k * BS + ch * CB:k * BS + ch * CB + CB, tau,
                        off:off + TGT] = blk.T
    return out, res.exec_time_ns


def kernel(**inputs):
    out, _ = run(inputs, trace=False)
    return out


# revision 23
# speedup vs baseline: 1.2031x; 1.0340x over previous
"""DeepAR (2-layer LSTM, HID=128) forward on 8 Trainium2 NeuronCores.

Sharding: pure data parallelism. Batch 2048 -> 256 rows per core, run as two
128-row chains; LSTM weights replicated; no cross-device communication.

Device kernel ("ride-along sigmoids + fp8 DoubleRow recurrence"):
  - ACT is the bottleneck engine, so the kernel issues exactly TWO sigmoid
    instructions per chain per step and nothing else on ACT:
      I_A(t) = sigmoid[ g1(t) gates (4*128) | C2(t-2) ride (128) ]
      I_B(t) = sigmoid[ g2(t-1) gates      | C1(t) ride        ]
    Layer 2 runs one step behind layer 1. The cell state is kept doubled
    (C = 2c) so one sigmoid covers both the gates (g-rows of the weights are
    pre-doubled: tanh(g) = 2*sig(2g)-1) and the tanh rides
    (tanh(c) = 2*sig(C)-1). No standalone Tanh instructions exist.
  - The recurrent state enters the matmuls as m = h/2 = so*(sig(C)-0.5),
    one fused scalar_tensor_tensor on DVE, written directly as fp8.
    Each gate chunk is then ONE DoubleRow fp8 matmul with virtual K=256:
      layer1: rhs = [x(t) | m1(t-1)],   lhsT = [Wih0 | 2*Whh0]
      layer2: rhs = [m1(t-1) | m2(t-2)], lhsT = [2*Wih1 | 2*Whh1]
    x / m1 / m2 live as adjacent 128-col regions of one per-slot layout
    [x(s) | m1(s-1) | m2(s-2)] inside XCH-slot chunk tiles, so both layers'
    rhs pairs are contiguous 3D APs of the same tile, and the projection
    reads 4-slot m2 spans.
  - Cell update per layer on DVE (gt = 4*sig(2g)-2, u = si*gt, C = u+v) with
    v = sf*C_prev on GpSimd; the psum ride region is filled by two identity
    matmuls (I@u + I@v) on the tensor engine.
  - One 2-bank PSUM slot per chain holds I_A then I_B alternately (the
    pipeline's data deps already serialize the reuse); projection psum and
    staging run incrementally; means/sigma post-processing runs per 512-col
    block (means on GpSimd, softplus Exp/Ln on ACT) as blocks complete.

kernel(**inputs) is self-contained: hardcodes shapes, shards, compiles via
bass/Tile, runs on cores 0-7 through bass_utils.run_bass_kernel_spmd, and
reassembles the full [2048, 192, 14] float32 output.
"""

import numpy as np
import ml_dtypes

import concourse.bass as bass
import concourse.mybir as mybir
from concourse.tile import TileContext
from concourse import bacc, bass_utils

F32 = mybir.dt.float32
BF16 = mybir.dt.bfloat16
F8 = mybir.dt.float8e4
AF = mybir.ActivationFunctionType
ALU = mybir.AluOpType
DR = mybir.MatmulPerfMode.DoubleRow

f8 = ml_dtypes.float8_e4m3
bf = ml_dtypes.bfloat16

# Model dims (hardcoded from the problem spec)
B = 2048
SEQ = 168
PRED = 24
W = SEQ + PRED          # 192
TGT = 7
TNUM = 4
TCAT = 3
MNUM = 2
MCAT = 2
COV = 4
HID = 128
INP = 65                # 7 + 28 + 4 + 26
INPX = INP + 1          # + ones row for the L1 bias
NCORES = 8
BS = B // NCORES        # 256 batch rows per core
CB = 128                # chain batch (2 chains per core)
XCH = 28                # slots per chunk tile
GRP = 4                 # slots per projection group
PGRP = 16               # groups per packed partition block (16*7 = 112)

_CACHE = {}


def _plan(nsteps):
    slots = nsteps + 2                    # slot s: [x(s) | m1(s-1) | m2(s-2)]
    slots_pad = -(-slots // XCH) * XCH
    nchunks = slots_pad // XCH
    ngrp = slots_pad // GRP               # per-chain projection groups
    ngrp_tot = 2 * ngrp                   # global groups G = 2*g + ch
    nblk = -(-ngrp_tot // PGRP)           # 512-col blocks in the packed sb
    return slots, slots_pad, nchunks, ngrp, ngrp_tot, nblk


# --------------------------------------------------------------------------
# host-side preprocessing
# --------------------------------------------------------------------------

def _host_prep(inputs):
    ge = inputs["given_enc"].astype(np.float32)
    xe = inputs["x_enc"].astype(np.float32)
    xm = inputs["x_mark_enc"].astype(np.float32)
    mx = inputs["meta_x"].astype(np.float32)
    tembs = [inputs["temb0"], inputs["temb1"], inputs["temb2"]]
    membs = [inputs["memb0"], inputs["memb1"]]

    ge_cat = [tembs[i][ge[:, :, TNUM + i].astype(np.int32)] for i in range(TCAT)]
    mx_cat = [membs[i][mx[:, MNUM + i].astype(np.int32)] for i in range(MCAT)]

    norm_mean = xe.mean(axis=1, keepdims=True)                 # [B,1,7]
    xc = xe - norm_mean
    norm_std = np.sqrt((xc * xc).mean(axis=1, keepdims=True) + 1e-5)
    xn = xc / norm_std

    idx = np.clip(np.arange(W) - 1, 0, SEQ - 1)
    prev_y = xn[:, idx, :]                                     # [B,W,7]

    mx_embed = np.concatenate([mx[:, :MNUM]] + mx_cat, axis=-1)   # [B,26]
    mx_b = np.broadcast_to(mx_embed[:, None, :], (B, W, mx_embed.shape[-1]))
    inp = np.concatenate(
        [prev_y, ge[:, :, :TNUM]] + ge_cat + [xm, mx_b], axis=-1
    )                                                          # [B,W,65]
    return inp, norm_mean[:, 0, :], norm_std[:, 0, :]          # [B,7] stats


def _gscale(wT):
    """Double the g-gate block (cols 2H:3H of the gate dim). wT is [K, 4H]."""
    w = wT.copy()
    w[:, 2 * HID:3 * HID] *= 2.0
    return w


def _host_weights(inputs):
    w = {}
    # L1 DoubleRow weights [128, 2, 512]: half0 = Wih0^T (+bias row), half1 = 2*Whh0^T
    w1 = np.zeros((HID, 2, 4 * HID), np.float32)
    w1[:INPX, 0, :] = _gscale(np.concatenate(
        [inputs["Wih0"].T, (inputs["bih0"] + inputs["bhh0"])[None, :]], axis=0))
    w1[:, 1, :] = _gscale(2.0 * inputs["Whh0"].T)
    w["w1dr"] = w1.astype(f8)
    # L2 DoubleRow weights: half0 = 2*Wih1^T, half1 = 2*Whh1^T
    w2 = np.zeros((HID, 2, 4 * HID), np.float32)
    w2[:, 0, :] = _gscale(2.0 * inputs["Wih1"].T)
    w2[:, 1, :] = _gscale(2.0 * inputs["Whh1"].T)
    w["w2dr"] = w2.astype(f8)
    # L2 bias via indicator matmul
    b2 = _gscale((inputs["bih1"] + inputs["bhh1"])[None, :])[0]
    w["b2all"] = b2.reshape(4, HID).astype(bf)
    ind = np.zeros((4, 4 * HID), np.float32)
    for k in range(4):
        ind[k, HID * k:HID * (k + 1)] = 1.0
    w["ind"] = ind.astype(bf)
    # projection: h2 = 2*m2 -> lhsT = [2*Wm; 2*Ws]^T  [128, 14]
    w["wproj"] = (2.0 * np.concatenate([inputs["Wm"], inputs["Ws"]], axis=0).T
                  ).astype(bf)
    return w


# --------------------------------------------------------------------------
# device kernel builder
# --------------------------------------------------------------------------

def build_module(nsteps=W, no_proj=False):
    slots, slots_pad, nchunks, ngrp, ngrp_tot, nblk = _plan(nsteps)
    nc = bacc.Bacc("TRN2", target_bir_lowering=False, debug=False,
                   enable_asserts=False, num_devices=NCORES)

    xin_d = [nc.dram_tensor(f"xin{ch}", [INPX, nsteps * CB], F8,
                            kind="ExternalInput").ap() for ch in (0, 1)]
    w1dr_d = nc.dram_tensor("w1dr", [HID, 2 * 4 * HID], F8, kind="ExternalInput").ap()
    w2dr_d = nc.dram_tensor("w2dr", [HID, 2 * 4 * HID], F8, kind="ExternalInput").ap()
    b2all_d = nc.dram_tensor("b2all", [4, HID], BF16, kind="ExternalInput").ap()
    ind_d = nc.dram_tensor("ind", [4, 4 * HID], BF16, kind="ExternalInput").ap()
    wproj_d = nc.dram_tensor("wproj", [HID, 2 * TGT], BF16, kind="ExternalInput").ap()
    stdm_d = nc.dram_tensor("stdm", [TGT * PGRP, 512 * nblk], F32,
                            kind="ExternalInput").ap()
    madd_d = nc.dram_tensor("madd", [TGT * PGRP, 512 * nblk], F32,
                            kind="ExternalInput").ap()
    bsp_d = nc.dram_tensor("bsp", [TGT * PGRP, 1], F32, kind="ExternalInput").ap()

    means_d = nc.dram_tensor("means", [TGT * PGRP, 512 * nblk], F32,
                             kind="ExternalOutput").ap()
    sigmas_d = nc.dram_tensor("sigmas", [TGT * PGRP, 512 * nblk], F32,
                              kind="ExternalOutput").ap()

    with TileContext(nc) as tc:
        import contextlib
        ctx = contextlib.ExitStack()
        singles = ctx.enter_context(tc.tile_pool(name="singles", bufs=1))
        xmp = {ch: ctx.enter_context(tc.tile_pool(name=f"xm{ch}", bufs=3))
               for ch in (0, 1)}
        sap = {ch: ctx.enter_context(tc.tile_pool(name=f"sa{ch}", bufs=2))
               for ch in (0, 1)}
        sbp = {ch: ctx.enter_context(tc.tile_pool(name=f"sb{ch}", bufs=2))
               for ch in (0, 1)}
        vecp = ctx.enter_context(tc.tile_pool(name="vec", bufs=2))
        pgp = {ch: ctx.enter_context(
            tc.tile_pool(name=f"pg{ch}", bufs=1, space="PSUM"))
            for ch in (0, 1)}
        projp = ctx.enter_context(tc.tile_pool(name="proj", bufs=2, space="PSUM"))
        stagep = ctx.enter_context(tc.tile_pool(name="stage", bufs=3))

        def load(name, dram, shape, dtype=BF16):
            t = singles.tile(shape, dtype, tag=name, name=name)
            nc.sync.dma_start(out=t[:], in_=dram)
            return t

        from concourse.masks import make_identity
        ident = singles.tile([HID, HID], BF16, tag="ident", name="ident")
        make_identity(nc, ident[:])
        # 4*I: rides feed u/4, the matmul scales it back (saves a DVE hop)
        ident4 = singles.tile([HID, HID], BF16, tag="ident4", name="ident4")
        nc.vector.tensor_scalar(ident4[:], ident[:], 4.0, None, ALU.mult)
        w1dr = load("w1dr", w1dr_d, [HID, 2, 4 * HID], F8)
        w2dr = load("w2dr", w2dr_d, [HID, 2, 4 * HID], F8)
        b2all = load("b2all", b2all_d, [4, HID])
        ind = load("ind", ind_d, [4, 4 * HID])
        wproj = load("wproj", wproj_d, [HID, 2 * TGT])
        stdm = load("stdm", stdm_d, [TGT * PGRP, 512 * nblk], F32)
        madd = load("madd", madd_d, [TGT * PGRP, 512 * nblk], F32)
        bsp = load("bsp", bsp_d, [TGT * PGRP, 1], F32)

        means_sb = singles.tile([TGT * PGRP, 512 * nblk], F32, tag="means_sb",
                                name="means_sb")
        sigraw_sb = singles.tile([TGT * PGRP, 512 * nblk], F32, tag="sigraw_sb",
                                 name="sigraw_sb")

        # chunk tiles: [128, XCH, 3, 128] fp8; slot s: [x(s) | m1(s-1) | m2(s-2)]
        chunk = {0: {}, 1: {}}

        def get_chunk(ch, ci):
            if ci not in chunk[ch]:
                xt = xmp[ch].tile([HID, XCH, 3, CB], F8, tag="xm",
                                  name=f"xm{ch}_{ci}")
                chunk[ch][ci] = xt
                if ci < 3:
                    # one-time per physical buffer: zero the x rows the DMA
                    # never writes (their weights are zero, but stale NaNs
                    # would still poison the fp8 matmul). Partition base must
                    # be 32-aligned; rows 64-65 get re-written by the DMA.
                    nc.gpsimd.memset(xt[64:, :, 0, :], 0.0)
                if ci == 0:
                    nc.gpsimd.memset(xt[:, 0, 1:3, :], 0.0)   # m1(-1), m2(-2)
                    nc.gpsimd.memset(xt[:, 1, 2, :], 0.0)     # m2(-1)
                if ci == nchunks - 1 and slots - ci * XCH < XCH:
                    nc.gpsimd.memset(xt[:, slots - ci * XCH:, 2, :], 0.0)
                s0 = ci * XCH
                nx = min(XCH, nsteps - s0)
                if nx > 0:
                    nc.sync.dma_start(
                        out=xt[:INPX, 0:nx, 0, :],
                        in_=xin_d[ch][:, s0 * CB:(s0 + nx) * CB])
            return chunk[ch][ci]

        def slot(ch, s):
            return get_chunk(ch, s // XCH)[:, s % XCH, :, :]

        state = [dict(c1=None, c2=None, sA=None, sB=None) for _ in (0, 1)]
        pg = {ch: pgp[ch].tile([HID, 640], F32, tag="pg", name=f"pg{ch}")
              for ch in (0, 1)}

        # ---- projection / staging / epilogue ----
        commit_q = []
        grp_done = [0, 0]
        blk_parts = [0] * nblk
        pool_q = []          # deferred small gpsimd ops: (fn,) chopped pieces

        def emit_block(cc):
            """Means post-processing for a finished 512-col block, chopped
            into small gpsimd pieces so the cell's v-muls never queue behind
            a long block op. Sigma softplus is deferred to the tail."""
            d0 = 512 * cc
            for p in range(4):
                d = slice(d0 + 128 * p, d0 + 128 * (p + 1))
                pool_q.append(lambda d=d: nc.gpsimd.tensor_tensor(
                    means_sb[:, d], means_sb[:, d], stdm[:, d], ALU.mult))
                pool_q.append(lambda d=d: nc.gpsimd.tensor_tensor(
                    means_sb[:, d], means_sb[:, d], madd[:, d], ALU.add))
            dd = slice(d0, d0 + 512)
            pool_q.append(lambda dd=dd: nc.sync.dma_start(
                out=means_d[:, dd], in_=means_sb[:, dd]))

        def drain_pool_q(k=2):
            for _ in range(min(k, len(pool_q))):
                pool_q.pop(0)()

        def emit_group(ch, g):
            """Project slots [4g, 4g+4) of chain ch, stage + pack."""
            G = 2 * g + ch
            ci = (GRP * g) // XCH
            xt = chunk[ch][ci]
            so = (GRP * g) % XCH
            pp = projp.tile([2 * TGT, 512], F32, tag="pp", name=f"pp{G}")
            nc.tensor.matmul(pp[:], wproj[:], xt[:, so:so + GRP, 2, :],
                             start=True, stop=True)
            st = stagep.tile([2 * TGT, 512], F32, tag="st", name=f"st{G}")
            nc.vector.tensor_copy(st[:], pp[:])
            prow = TGT * (G % PGRP)
            cc = G // PGRP
            dst = slice(512 * cc, 512 * (cc + 1))
            nc.sync.dma_start(out=means_sb[prow:prow + TGT, dst], in_=st[:TGT, :])
            nc.sync.dma_start(out=sigraw_sb[prow:prow + TGT, dst], in_=st[TGT:, :])
            blk_parts[cc] += 1
            if blk_parts[cc] == min(PGRP, ngrp_tot - cc * PGRP):
                emit_block(cc)

        def maybe_proj(ch, t):
            if no_proj:
                return
            # group g's last m2 (slot 4g+3) is written during tick 4g+3
            while grp_done[ch] < ngrp and t >= GRP * grp_done[ch] + GRP - 1:
                emit_group(ch, grp_done[ch])
                grp_done[ch] += 1

        def tail_sigma():
            """Bulk softplus + denorm of the packed sigma rows, then DMA."""
            n = 512 * nblk
            h = n // 2
            for d in (slice(0, h), slice(h, n)):
                nc.scalar.activation(sigraw_sb[:, d], sigraw_sb[:, d], AF.Exp,
                                     bias=bsp[:, :])
                nc.scalar.activation(sigraw_sb[:, d], sigraw_sb[:, d], AF.Ln,
                                     bias=1.0)
                nc.vector.tensor_mul(sigraw_sb[:, d], sigraw_sb[:, d],
                                     stdm[:, d])
                nc.sync.dma_start(out=sigmas_d[:, d], in_=sigraw_sb[:, d])

        # ---- per-step pieces ----
        def cell(ch, layer, sgt, first):
            """Cell update. u4 = u/4 = (sig(2g)-0.5)*si comes straight off the
            sigma outputs (no chained DVE ops on the ride path); v on GpSimd;
            C = 4*u4 + v in one fused op off the critical path."""
            st = state[ch]
            si, sf = sgt[:, 0:CB], sgt[:, CB:2 * CB]
            s2g = sgt[:, 2 * CB:3 * CB]
            u4 = vecp.tile([HID, CB], BF16, tag=f"u{layer}{ch}",
                           name=f"u{layer}{ch}")
            nc.vector.scalar_tensor_tensor(u4[:], s2g, 0.5, si,
                                           ALU.subtract, ALU.mult)
            if first:
                v = None
            else:
                v = vecp.tile([HID, CB], BF16, tag=f"v{layer}{ch}",
                              name=f"v{layer}{ch}")
                nc.gpsimd.tensor_tensor(v[:], sf, st[f"c{layer}"][:], ALU.mult)
            # C = 4*u4 + v is deferred to the tick end (slack: only the next
            # tick's v-mul reads it) so it never delays the next m/u4 pair
            commit_q.append((ch, layer, u4, v))
            return u4, v

        def cell_commit():
            while commit_q:
                ch, layer, u4, v = commit_q.pop(0)
                c_new = vecp.tile([HID, CB], BF16, tag=f"c{layer}{ch}",
                                  name=f"c{layer}{ch}")
                if v is None:
                    nc.vector.tensor_scalar(c_new[:], u4[:], 4.0, None,
                                            ALU.mult)
                else:
                    nc.vector.scalar_tensor_tensor(c_new[:], u4[:], 4.0, v[:],
                                                   ALU.mult, ALU.add)
                state[ch][f"c{layer}"] = c_new

        def ride(ch, u4, v, lo=512):
            nc.tensor.matmul(pg[ch][:, lo:lo + CB], ident4[:], u4[:],
                             start=True, stop=(v is None))
            if v is not None:
                nc.tensor.matmul(pg[ch][:, lo:lo + CB], ident[:], v[:],
                                 start=False, stop=True)

        def m_stt(dst, s_c, s_o):
            """dst(fp8) = (sig(C) - 0.5) * so   [= h/2]"""
            nc.vector.scalar_tensor_tensor(dst, s_c, 0.5, s_o,
                                           ALU.subtract, ALU.mult)

        def l2_gates(ch, t):
            """bias + 4 DoubleRow matmuls for g2(t-1) into pg[ch][0:512]."""
            sl = slot(ch, t)
            nc.tensor.matmul(pg[ch][:, 0:512], b2all[:], ind[:],
                             start=True, stop=False)
            for m in range(4):
                nc.tensor.matmul(pg[ch][:, CB * m:CB * (m + 1)],
                                 w2dr[:, :, HID * m:HID * (m + 1)],
                                 sl[:, 1:3, :], start=False, stop=True,
                                 perf_mode=DR)

        def l1_gates(ch, t):
            """4 DoubleRow matmuls for g1(t) into pg[ch][0:512]."""
            get_chunk(ch, min((t + 4) // XCH, nchunks - 1))
            sl = slot(ch, t)
            for m in range(4):
                nc.tensor.matmul(pg[ch][:, CB * m:CB * (m + 1)],
                                 w1dr[:, :, HID * m:HID * (m + 1)],
                                 sl[:, 0:2, :], start=True, stop=True,
                                 perf_mode=DR)

        # ---------------- main loop ----------------
        # prologue: gate matmuls for g1(0) (later ticks emit them in half B)
        for ch in (0, 1):
            l1_gates(ch, 0)

        for t in range(nsteps + 1):
            # ---- half A ----
            for ch in (0, 1):
                st = state[ch]
                if t < nsteps:
                    hi = 640 if t >= 2 else 512
                    sA = sap[ch].tile([HID, 640], BF16, tag="sA",
                                      name=f"sA{ch}_{t}")
                    nc.scalar.activation(sA[:, :hi], pg[ch][:, :hi], AF.Sigmoid)
                    if t >= 2:
                        # m2(t-2) -> slot t region 2 ; so2(t-2) is in sB(t-1)
                        m_stt(slot(ch, t)[:, 2, :], sA[:, 512:640],
                              st["sB"][:, 3 * CB:4 * CB])
                    if t >= 1:
                        l2_gates(ch, t)       # PE: ready before the rides
                    u1, v1 = cell(ch, 1, sA, t == 0)
                    st["sA"] = sA
                    ride(ch, u1, v1)
                else:
                    # tail: the C2(nsteps-2) ride was written at t-1 half B;
                    # sigma it alone, then the last L2 gates
                    sA = sap[ch].tile([HID, 640], BF16, tag="sA",
                                      name=f"sA{ch}_{t}")
                    nc.scalar.activation(sA[:, 512:640], pg[ch][:, 512:640],
                                         AF.Sigmoid)
                    m_stt(slot(ch, t)[:, 2, :], sA[:, 512:640],
                          st["sB"][:, 3 * CB:4 * CB])
                    st["sA"] = sA
                    l2_gates(ch, t)

            # ---- half B ----
            for ch in (0, 1):
                st = state[ch]
                sB = sbp[ch].tile([HID, 640], BF16, tag="sB", name=f"sB{ch}_{t}")
                if t == 0:
                    nc.scalar.activation(sB[:, 512:640], pg[ch][:, 512:640],
                                         AF.Sigmoid)
                elif t == nsteps:
                    nc.scalar.activation(sB[:, 0:512], pg[ch][:, 0:512],
                                         AF.Sigmoid)
                else:
                    nc.scalar.activation(sB[:], pg[ch][:], AF.Sigmoid)
                if t < nsteps:
                    # m1(t) -> slot t+1 region 1 ; so1(t) is in sA(t)
                    m_stt(slot(ch, t + 1)[:, 1, :], sB[:, 512:640],
                          st["sA"][:, 3 * CB:4 * CB])
                    l1_gates(ch, t + 1)       # PE: before the rides
                if t >= 1:
                    u2, v2 = cell(ch, 2, sB, t == 1)
                    ride(ch, u2, v2)   # C2(t-1) ride -> next sigma's region
                st["sB"] = sB

            # ---- deferred/slack work at tick end ----
            cell_commit()
            for ch in (0, 1):
                maybe_proj(ch, t)
            drain_pool_q()

            if t == nsteps:
                # final ride: sigma C2(nsteps-1), emit m2(nsteps-1), flush
                for ch in (0, 1):
                    st = state[ch]
                    sF = sap[ch].tile([HID, 640], BF16, tag="sA",
                                      name=f"sF{ch}")
                    nc.scalar.activation(sF[:, 512:640], pg[ch][:, 512:640],
                                         AF.Sigmoid)
                    m_stt(slot(ch, t + 1)[:, 2, :], sF[:, 512:640],
                          st["sB"][:, 3 * CB:4 * CB])
                    maybe_proj(ch, 10 ** 9)
                drain_pool_q(10 ** 9)
                if not no_proj:
                    tail_sigma()

        ctx.close()

    nc.finalize()
    return nc


# --------------------------------------------------------------------------
# top-level entry
# --------------------------------------------------------------------------

def run(inputs, trace=False, nsteps=W):
    inputs = {k: np.asarray(v) for k, v in inputs.items()}
    slots, slots_pad, nchunks, ngrp, ngrp_tot, nblk = _plan(nsteps)
    inp, nmean, nstd = _host_prep(inputs)
    wts = _host_weights(inputs)
    bm = inputs["bm"].astype(np.float32)
    bs_ = inputs["bs"].astype(np.float32)

    in_maps = []
    for k in range(NCORES):
        m = {kk: (vv.reshape(vv.shape[0], -1) if vv.ndim == 3 else vv)
             for kk, vv in wts.items()}
        for ch in (0, 1):
            bsl = slice(k * BS + ch * CB, k * BS + ch * CB + CB)
            xi = np.concatenate(
                [inp[bsl, :nsteps, :], np.ones((CB, nsteps, 1), np.float32)],
                axis=-1)                                   # [CB, T, 66]
            m[f"xin{ch}"] = np.ascontiguousarray(
                xi.transpose(2, 1, 0).reshape(INPX, -1)).astype(f8)
        stdm = np.zeros((TGT * PGRP, 512 * nblk), np.float32)
        madd = np.zeros((TGT * PGRP, 512 * nblk), np.float32)
        for G in range(ngrp_tot):
            ch = G % 2
            prow = TGT * (G % PGRP)
            c0 = 512 * (G // PGRP)
            bsl = slice(k * BS + ch * CB, k * BS + ch * CB + CB)
            std_c = nstd[bsl]      # [CB, 7]
            mean_c = nmean[bsl]
            for j in range(GRP):
                cs = slice(c0 + CB * j, c0 + CB * (j + 1))
                stdm[prow:prow + TGT, cs] = std_c.T
                madd[prow:prow + TGT, cs] = bm[:, None] * std_c.T + mean_c.T
        m["stdm"] = stdm
        m["madd"] = madd
        m["bsp"] = np.tile(bs_, PGRP)[:, None].astype(np.float32)
        in_maps.append(m)

    key = nsteps
    if key not in _CACHE:
        _CACHE[key] = build_module(nsteps)
    nc = _CACHE[key]

    res = bass_utils.run_bass_kernel_spmd(
        nc, in_maps, core_ids=list(range(NCORES)), trace=False)

    out = np.empty((B, nsteps, 2 * TGT), np.float32)
    for k in range(NCORES):
        r = res.results[k]
        for name, off in (("means", 0), ("sigmas", TGT)):
            a = r[name]                      # [112, 512*nblk]
            for G in range(ngrp_tot):
                ch, g = G % 2, G // 2
                prow = TGT * (G % PGRP)
                c0 = 512 * (G // PGRP)
                for j in range(GRP):
                    tau = GRP * g + j - 2
                    if tau < 0 or tau >= nsteps:
                        continue
                    blk = a[prow:prow + TGT, c0 + CB * j:c0 + CB * (j + 1)]
                    out[k * BS + ch * CB:k * BS + ch * CB + CB, tau,
                        off:off + TGT] = blk.T
    return out, res.exec_time_ns


def kernel(**inputs):
    out, _ = run(inputs, trace=False)
    return out


# revision 25
# speedup vs baseline: 1.2307x; 1.0229x over previous
"""DeepAR forward on 8 TRN2 cores — 3-chain variant (96/96/64 batch split).

Same architecture as kernel.py (ride-along sigmoids, L2 lagged, fp8 DoubleRow
recurrence, m = h/2 state) but the 256-row shard runs as THREE independent
recurrence chains. ACT busy is invariant to the split (sum of gate elements
fixed); per-chain loop latency drops below ACT busy, so the wall becomes the
ACT engine instead of the dependency loop.
"""

import numpy as np
import ml_dtypes

import concourse.bass as bass
import concourse.mybir as mybir
from concourse.tile import TileContext
from concourse import bacc, bass_utils

F32 = mybir.dt.float32
BF16 = mybir.dt.bfloat16
F8 = mybir.dt.float8e4
AF = mybir.ActivationFunctionType
ALU = mybir.AluOpType
DR = mybir.MatmulPerfMode.DoubleRow

f8 = ml_dtypes.float8_e4m3
bf = ml_dtypes.bfloat16

B = 2048
SEQ = 168
PRED = 24
W = SEQ + PRED          # 192
TGT = 7
TNUM = 4
TCAT = 3
MNUM = 2
MCAT = 2
COV = 4
HID = 128
INP = 65
INPX = INP + 1
NCORES = 8
BS = B // NCORES        # 256
CBS = (96, 96, 64)      # per-chain batch
COFF = (0, 96, 192)
CHS = (0, 1, 2)
XCH = 28                # slots per chunk tile
GRP = 4                 # slots per projection group
PGRP = 16               # groups per packed partition block (16*7 = 112)

_CACHE = {}


def _plan(nsteps):
    slots = nsteps + 2
    slots_pad = -(-slots // XCH) * XCH
    nchunks = slots_pad // XCH
    ngrp = slots_pad // GRP                 # per-chain projection groups
    nblk = -(-ngrp // PGRP)                 # blocks per chain
    gw = [4 * cb for cb in CBS]             # block col width per chain
    cbase = [0, nblk * gw[0], nblk * (gw[0] + gw[1])]
    ncol = nblk * sum(gw)
    return slots, slots_pad, nchunks, ngrp, nblk, gw, cbase, ncol


def _host_prep(inputs):
    ge = inputs["given_enc"].astype(np.float32)
    xe = inputs["x_enc"].astype(np.float32)
    xm = inputs["x_mark_enc"].astype(np.float32)
    mx = inputs["meta_x"].astype(np.float32)
    tembs = [inputs["temb0"], inputs["temb1"], inputs["temb2"]]
    membs = [inputs["memb0"], inputs["memb1"]]
    ge_cat = [tembs[i][ge[:, :, TNUM + i].astype(np.int32)] for i in range(TCAT)]
    mx_cat = [membs[i][mx[:, MNUM + i].astype(np.int32)] for i in range(MCAT)]
    norm_mean = xe.mean(axis=1, keepdims=True)
    xc = xe - norm_mean
    norm_std = np.sqrt((xc * xc).mean(axis=1, keepdims=True) + 1e-5)
    xn = xc / norm_std
    idx = np.clip(np.arange(W) - 1, 0, SEQ - 1)
    prev_y = xn[:, idx, :]
    mx_embed = np.concatenate([mx[:, :MNUM]] + mx_cat, axis=-1)
    mx_b = np.broadcast_to(mx_embed[:, None, :], (B, W, mx_embed.shape[-1]))
    inp = np.concatenate(
        [prev_y, ge[:, :, :TNUM]] + ge_cat + [xm, mx_b], axis=-1)
    return inp, norm_mean[:, 0, :], norm_std[:, 0, :]


def _gscale(wT):
    w = wT.copy()
    w[:, 2 * HID:3 * HID] *= 2.0
    return w


def _host_weights(inputs):
    w = {}
    w1 = np.zeros((HID, 2, 4 * HID), np.float32)
    w1[:INPX, 0, :] = _gscale(np.concatenate(
        [inputs["Wih0"].T, (inputs["bih0"] + inputs["bhh0"])[None, :]], axis=0))
    w1[:, 1, :] = _gscale(2.0 * inputs["Whh0"].T)
    w["w1dr"] = w1.astype(f8)
    w2 = np.zeros((HID, 2, 4 * HID), np.float32)
    w2[:, 0, :] = _gscale(2.0 * inputs["Wih1"].T)
    w2[:, 1, :] = _gscale(2.0 * inputs["Whh1"].T)
    w["w2dr"] = w2.astype(f8)
    b2 = _gscale((inputs["bih1"] + inputs["bhh1"])[None, :])[0]
    w["b2all"] = b2.reshape(4, HID).astype(bf)
    for ch in CHS:
        cb = CBS[ch]
        ind = np.zeros((4, 4 * cb), np.float32)
        for k in range(4):
            ind[k, cb * k:cb * (k + 1)] = 1.0
        w[f"ind{ch}"] = ind.astype(bf)
    w["wproj"] = (2.0 * np.concatenate([inputs["Wm"], inputs["Ws"]], axis=0).T
                  ).astype(bf)
    return w


def build_module(nsteps=W):
    slots, slots_pad, nchunks, ngrp, nblk, gw, cbase, ncol = _plan(nsteps)
    nc = bacc.Bacc("TRN2", target_bir_lowering=False, debug=False,
                   enable_asserts=False, num_devices=NCORES)

    xin_d = [nc.dram_tensor(f"xin{ch}", [INPX, nsteps * CBS[ch]], F8,
                            kind="ExternalInput").ap() for ch in CHS]
    w1dr_d = nc.dram_tensor("w1dr", [HID, 2 * 4 * HID], F8, kind="ExternalInput").ap()
    w2dr_d = nc.dram_tensor("w2dr", [HID, 2 * 4 * HID], F8, kind="ExternalInput").ap()
    b2all_d = nc.dram_tensor("b2all", [4, HID], BF16, kind="ExternalInput").ap()
    ind_d = [nc.dram_tensor(f"ind{ch}", [4, 4 * CBS[ch]], BF16,
                            kind="ExternalInput").ap() for ch in CHS]
    wproj_d = nc.dram_tensor("wproj", [HID, 2 * TGT], BF16, kind="ExternalInput").ap()
    stdm_d = nc.dram_tensor("stdm", [TGT * PGRP, ncol], F32,
                            kind="ExternalInput").ap()
    madd_d = nc.dram_tensor("madd", [TGT * PGRP, ncol], F32,
                            kind="ExternalInput").ap()
    bsp_d = nc.dram_tensor("bsp", [TGT * PGRP, 1], F32, kind="ExternalInput").ap()
    means_d = nc.dram_tensor("means", [TGT * PGRP, ncol], F32,
                             kind="ExternalOutput").ap()
    sigmas_d = nc.dram_tensor("sigmas", [TGT * PGRP, ncol], F32,
                              kind="ExternalOutput").ap()

    with TileContext(nc) as tc:
        import contextlib
        ctx = contextlib.ExitStack()
        singles = ctx.enter_context(tc.tile_pool(name="singles", bufs=1))
        xmp = {ch: ctx.enter_context(tc.tile_pool(name=f"xm{ch}", bufs=3))
               for ch in CHS}
        sap = {ch: ctx.enter_context(tc.tile_pool(name=f"sa{ch}", bufs=2))
               for ch in CHS}
        sbp = {ch: ctx.enter_context(tc.tile_pool(name=f"sb{ch}", bufs=2))
               for ch in CHS}
        vecp = ctx.enter_context(tc.tile_pool(name="vec", bufs=2))
        pgp = {ch: ctx.enter_context(
            tc.tile_pool(name=f"pg{ch}", bufs=1, space="PSUM")) for ch in CHS}
        projp = {ch: ctx.enter_context(
            tc.tile_pool(name=f"proj{ch}", bufs=1, space="PSUM")) for ch in CHS}
        stagep = ctx.enter_context(tc.tile_pool(name="stage", bufs=3))

        def load(name, dram, shape, dtype=BF16):
            t = singles.tile(shape, dtype, tag=name, name=name)
            nc.sync.dma_start(out=t[:], in_=dram)
            return t

        from concourse.masks import make_identity
        ident = singles.tile([HID, HID], BF16, tag="ident", name="ident")
        make_identity(nc, ident[:])
        ident4 = singles.tile([HID, HID], BF16, tag="ident4", name="ident4")
        nc.vector.tensor_scalar(ident4[:], ident[:], 4.0, None, ALU.mult)
        w1dr = load("w1dr", w1dr_d, [HID, 2, 4 * HID], F8)
        w2dr = load("w2dr", w2dr_d, [HID, 2, 4 * HID], F8)
        b2all = load("b2all", b2all_d, [4, HID])
        ind = {ch: load(f"ind{ch}", ind_d[ch], [4, 4 * CBS[ch]]) for ch in CHS}
        wproj = load("wproj", wproj_d, [HID, 2 * TGT])
        stdm = load("stdm", stdm_d, [TGT * PGRP, ncol], F32)
        madd = load("madd", madd_d, [TGT * PGRP, ncol], F32)
        bsp = load("bsp", bsp_d, [TGT * PGRP, 1], F32)

        means_sb = singles.tile([TGT * PGRP, ncol], F32, tag="means_sb",
                                name="means_sb")
        sigraw_sb = singles.tile([TGT * PGRP, ncol], F32, tag="sigraw_sb",
                                 name="sigraw_sb")

        chunk = {ch: {} for ch in CHS}

        def get_chunk(ch, ci):
            cb = CBS[ch]
            if ci not in chunk[ch]:
                xt = xmp[ch].tile([HID, XCH, 3, cb], F8, tag="xm",
                                  name=f"xm{ch}_{ci}")
                chunk[ch][ci] = xt
                if ci < 3:
                    nc.gpsimd.memset(xt[64:, :, 0, :], 0.0)
                if ci == 0:
                    nc.gpsimd.memset(xt[:, 0, 1:3, :], 0.0)
                    nc.gpsimd.memset(xt[:, 1, 2, :], 0.0)
                if ci == nchunks - 1 and slots - ci * XCH < XCH:
                    nc.gpsimd.memset(xt[:, slots - ci * XCH:, 2, :], 0.0)
                s0 = ci * XCH
                nx = min(XCH, nsteps - s0)
                if nx > 0:
                    nc.sync.dma_start(
                        out=xt[:INPX, 0:nx, 0, :],
                        in_=xin_d[ch][:, s0 * cb:(s0 + nx) * cb])
            return chunk[ch][ci]

        def slot(ch, s):
            return get_chunk(ch, s // XCH)[:, s % XCH, :, :]

        state = [dict(c1=None, c2=None, sA=None, sB=None) for _ in CHS]
        pg = {ch: pgp[ch].tile([HID, 5 * CBS[ch]], F32, tag="pg",
                               name=f"pg{ch}") for ch in CHS}

        commit_q = []
        grp_done = [0, 0, 0]
        blk_parts = {}
        pool_q = []

        def emit_block(ch, cc):
            d0 = cbase[ch] + cc * gw[ch]
            for p in range(gw[ch] // 128):
                d = slice(d0 + 128 * p, d0 + 128 * (p + 1))
                pool_q.append(lambda d=d: nc.vector.tensor_tensor(
                    means_sb[:, d], means_sb[:, d], stdm[:, d], ALU.mult))
                pool_q.append(lambda d=d: nc.vector.tensor_tensor(
                    means_sb[:, d], means_sb[:, d], madd[:, d], ALU.add))
            dd = slice(d0, d0 + gw[ch])
            pool_q.append(lambda dd=dd: nc.sync.dma_start(
                out=means_d[:, dd], in_=means_sb[:, dd]))

        def drain_pool_q(k=2):
            for _ in range(min(k, len(pool_q))):
                pool_q.pop(0)()

        def emit_group(ch, g):
            cb = CBS[ch]
            ci = (GRP * g) // XCH
            xt = chunk[ch][ci]
            so = (GRP * g) % XCH
            pp = projp[ch].tile([2 * TGT, 4 * cb], F32, tag="pp",
                                name=f"pp{ch}_{g}")
            nc.tensor.matmul(pp[:], wproj[:], xt[:, so:so + GRP, 2, :],
                             start=True, stop=True)
            st = stagep.tile([2 * TGT, 4 * cb], F32, tag=f"st{ch}",
                             name=f"st{ch}_{g}")
            nc.vector.tensor_copy(st[:], pp[:])
            prow = TGT * (g % PGRP)
            cc = g // PGRP
            dst = slice(cbase[ch] + cc * gw[ch], cbase[ch] + cc * gw[ch] + gw[ch])
            nc.sync.dma_start(out=means_sb[prow:prow + TGT, dst], in_=st[:TGT, :])
            nc.sync.dma_start(out=sigraw_sb[prow:prow + TGT, dst], in_=st[TGT:, :])
            blk_parts[(ch, cc)] = blk_parts.get((ch, cc), 0) + 1
            if blk_parts[(ch, cc)] == min(PGRP, ngrp - cc * PGRP):
                emit_block(ch, cc)

        def maybe_proj(ch, t):
            while grp_done[ch] < ngrp and t >= GRP * grp_done[ch] + GRP - 1:
                emit_group(ch, grp_done[ch])
                grp_done[ch] += 1

        def tail_sigma():
            h = ncol // 2
            for d in (slice(0, h), slice(h, ncol)):
                nc.scalar.activation(sigraw_sb[:, d], sigraw_sb[:, d], AF.Exp,
                                     bias=bsp[:, :])
                nc.scalar.activation(sigraw_sb[:, d], sigraw_sb[:, d], AF.Ln,
                                     bias=1.0)
                nc.vector.tensor_mul(sigraw_sb[:, d], sigraw_sb[:, d],
                                     stdm[:, d])
                nc.sync.dma_start(out=sigmas_d[:, d], in_=sigraw_sb[:, d])

        def cell(ch, layer, sgt, first):
            st = state[ch]
            cb = CBS[ch]
            si, sf = sgt[:, 0:cb], sgt[:, cb:2 * cb]
            s2g = sgt[:, 2 * cb:3 * cb]
            u4 = vecp.tile([HID, cb], BF16, tag=f"u{layer}{ch}",
                           name=f"u{layer}{ch}")
            nc.vector.scalar_tensor_tensor(u4[:], s2g, 0.5, si,
                                           ALU.subtract, ALU.mult)
            if first:
                v = None
            else:
                v = vecp.tile([HID, cb], BF16, tag=f"v{layer}{ch}",
                              name=f"v{layer}{ch}")
                nc.gpsimd.tensor_tensor(v[:], sf, st[f"c{layer}"][:], ALU.mult)
            commit_q.append((ch, layer, u4, v))
            return u4, v

        def cell_commit(k=10 ** 9):
            n = 0
            while commit_q and n < k:
                n += 1
                ch, layer, u4, v = commit_q.pop(0)
                c_new = vecp.tile([HID, CBS[ch]], BF16, tag=f"c{layer}{ch}",
                                  name=f"c{layer}{ch}")
                if v is None:
                    nc.vector.tensor_scalar(c_new[:], u4[:], 4.0, None,
                                            ALU.mult)
                elif ch == 1:
                    nc.vector.scalar_tensor_tensor(c_new[:], u4[:], 4.0, v[:],
                                                   ALU.mult, ALU.add)
                else:
                    nc.gpsimd.tensor_scalar(c_new[:], u4[:], 4.0, None,
                                            ALU.mult)
                    nc.gpsimd.tensor_tensor(c_new[:], c_new[:], v[:], ALU.add)
                state[ch][f"c{layer}"] = c_new

        def ride(ch, u4, v):
            cb = CBS[ch]
            lo = 4 * cb
            nc.tensor.matmul(pg[ch][:, lo:lo + cb], ident4[:], u4[:],
                             start=True, stop=(v is None))
            if v is not None:
                nc.tensor.matmul(pg[ch][:, lo:lo + cb], ident[:], v[:],
                                 start=False, stop=True)

        def m_stt(dst, s_c, s_o):
            nc.vector.scalar_tensor_tensor(dst, s_c, 0.5, s_o,
                                           ALU.subtract, ALU.mult)

        def l2_gates(ch, t):
            cb = CBS[ch]
            sl = slot(ch, t)
            nc.tensor.matmul(pg[ch][:, 0:4 * cb], b2all[:], ind[ch][:],
                             start=True, stop=False)
            for m in range(4):
                nc.tensor.matmul(pg[ch][:, cb * m:cb * (m + 1)],
                                 w2dr[:, :, HID * m:HID * (m + 1)],
                                 sl[:, 1:3, :], start=False, stop=True,
                                 perf_mode=DR)

        def l1_gates(ch, t):
            cb = CBS[ch]
            get_chunk(ch, min((t + 4) // XCH, nchunks - 1))
            sl = slot(ch, t)
            for m in range(4):
                nc.tensor.matmul(pg[ch][:, cb * m:cb * (m + 1)],
                                 w1dr[:, :, HID * m:HID * (m + 1)],
                                 sl[:, 0:2, :], start=True, stop=True,
                                 perf_mode=DR)

        # ---------------- main loop ----------------
        for ch in CHS:
            l1_gates(ch, 0)

        for t in range(nsteps + 1):
            # ---- half A ----
            for ch in CHS:
                st = state[ch]
                cb = CBS[ch]
                if t < nsteps:
                    hi = 5 * cb if t >= 2 else 4 * cb
                    sA = sap[ch].tile([HID, 5 * cb], BF16, tag="sA",
                                      name=f"sA{ch}_{t}")
                    nc.scalar.activation(sA[:, :hi], pg[ch][:, :hi], AF.Sigmoid)
                    if t >= 2:
                        m_stt(slot(ch, t)[:, 2, :], sA[:, 4 * cb:5 * cb],
                              st["sB"][:, 3 * cb:4 * cb])
                    if t >= 1:
                        l2_gates(ch, t)
                    u1, v1 = cell(ch, 1, sA, t == 0)
                    st["sA"] = sA
                    ride(ch, u1, v1)
                    cell_commit(1)
                else:
                    sA = sap[ch].tile([HID, 5 * cb], BF16, tag="sA",
                                      name=f"sA{ch}_{t}")
                    nc.scalar.activation(sA[:, 4 * cb:5 * cb],
                                         pg[ch][:, 4 * cb:5 * cb], AF.Sigmoid)
                    m_stt(slot(ch, t)[:, 2, :], sA[:, 4 * cb:5 * cb],
                          st["sB"][:, 3 * cb:4 * cb])
                    st["sA"] = sA
                    l2_gates(ch, t)

            # ---- half B ----
            for ch in CHS:
                st = state[ch]
                cb = CBS[ch]
                sB = sbp[ch].tile([HID, 5 * cb], BF16, tag="sB",
                                  name=f"sB{ch}_{t}")
                if t == 0:
                    nc.scalar.activation(sB[:, 4 * cb:5 * cb],
                                         pg[ch][:, 4 * cb:5 * cb], AF.Sigmoid)
                elif t == nsteps:
                    nc.scalar.activation(sB[:, 0:4 * cb], pg[ch][:, 0:4 * cb],
                                         AF.Sigmoid)
                else:
                    nc.scalar.activation(sB[:], pg[ch][:], AF.Sigmoid)
                if t < nsteps:
                    m_stt(slot(ch, t + 1)[:, 1, :], sB[:, 4 * cb:5 * cb],
                          st["sA"][:, 3 * cb:4 * cb])
                    l1_gates(ch, t + 1)
                if t >= 1:
                    u2, v2 = cell(ch, 2, sB, t == 1)
                    ride(ch, u2, v2)
                    cell_commit(1)
                st["sB"] = sB

            # ---- deferred/slack work at tick end ----
            cell_commit()
            for ch in CHS:
                maybe_proj(ch, t)
            drain_pool_q()

            if t == nsteps:
                for ch in CHS:
                    st = state[ch]
                    cb = CBS[ch]
                    sF = sap[ch].tile([HID, 5 * cb], BF16, tag="sA",
                                      name=f"sF{ch}")
                    nc.scalar.activation(sF[:, 4 * cb:5 * cb],
                                         pg[ch][:, 4 * cb:5 * cb], AF.Sigmoid)
                    m_stt(slot(ch, t + 1)[:, 2, :], sF[:, 4 * cb:5 * cb],
                          st["sB"][:, 3 * cb:4 * cb])
                    maybe_proj(ch, 10 ** 9)
                drain_pool_q(10 ** 9)
                tail_sigma()

        ctx.close()

    nc.finalize()
    return nc


def run(inputs, trace=False, nsteps=W):
    inputs = {k: np.asarray(v) for k, v in inputs.items()}
    slots, slots_pad, nchunks, ngrp, nblk, gw, cbase, ncol = _plan(nsteps)
    inp, nmean, nstd = _host_prep(inputs)
    wts = _host_weights(inputs)
    bm = inputs["bm"].astype(np.float32)
    bs_ = inputs["bs"].astype(np.float32)

    in_maps = []
    for k in range(NCORES):
        m = {kk: (vv.reshape(vv.shape[0], -1) if vv.ndim == 3 else vv)
             for kk, vv in wts.items()}
        for ch in CHS:
            cb = CBS[ch]
            bsl = slice(k * BS + COFF[ch], k * BS + COFF[ch] + cb)
            xi = np.concatenate(
                [inp[bsl, :nsteps, :], np.ones((cb, nsteps, 1), np.float32)],
                axis=-1)
            m[f"xin{ch}"] = np.ascontiguousarray(
                xi.transpose(2, 1, 0).reshape(INPX, -1)).astype(f8)
        stdm = np.zeros((TGT * PGRP, ncol), np.float32)
        madd = np.zeros((TGT * PGRP, ncol), np.float32)
        for ch in CHS:
            cb = CBS[ch]
            bsl = slice(k * BS + COFF[ch], k * BS + COFF[ch] + cb)
            std_c = nstd[bsl]
            mean_c = nmean[bsl]
            for g in range(ngrp):
                prow = TGT * (g % PGRP)
                c0 = cbase[ch] + (g // PGRP) * gw[ch]
                for j in range(GRP):
                    cs = slice(c0 + cb * j, c0 + cb * (j + 1))
                    stdm[prow:prow + TGT, cs] = std_c.T
                    madd[prow:prow + TGT, cs] = bm[:, None] * std_c.T + mean_c.T
        m["stdm"] = stdm
        m["madd"] = madd
        m["bsp"] = np.tile(bs_, PGRP)[:, None].astype(np.float32)
        in_maps.append(m)

    key = nsteps
    if key not in _CACHE:
        _CACHE[key] = build_module(nsteps)
    nc = _CACHE[key]

    res = bass_utils.run_bass_kernel_spmd(
        nc, in_maps, core_ids=list(range(NCORES)), trace=False)

    out = np.empty((B, nsteps, 2 * TGT), np.float32)
    for k in range(NCORES):
        r = res.results[k]
        for name, off in (("means", 0), ("sigmas", TGT)):
            a = r[name]
            for ch in CHS:
                cb = CBS[ch]
                b0 = k * BS + COFF[ch]
                for g in range(ngrp):
                    prow = TGT * (g % PGRP)
                    c0 = cbase[ch] + (g // PGRP) * gw[ch]
                    for j in range(GRP):
                        tau = GRP * g + j - 2
                        if tau < 0 or tau >= nsteps:
                            continue
                        blk = a[prow:prow + TGT, c0 + cb * j:c0 + cb * (j + 1)]
                        out[b0:b0 + cb, tau, off:off + TGT] = blk.T
    return out, res.exec_time_ns


def kernel(**inputs):
    out, _ = run(inputs, trace=False)
    return out
